# revision 1
# baseline (speedup 1.0000x reference)
"""Trainium2 Bass kernel for nn_Block_90890097918445 (dense transformer block).

Sharding: 8 cores. Tokens (B*T = 4096) split 512/core (batch-major) for
LN/QKV/proj/FFN; attention is head-parallel (2 heads/core over all 4096
tokens) which makes the causal work perfectly uniform across cores.
Two AllToAlls: (1) Q^T/K^T/V_aug head-redistribution after QKV,
(2) attention output back to token-parallel before the proj matmul.

All big matmuls run as float32r (full PE rate, ~1.7e-4 rel err).
Softmax skips max-subtraction (scores are provably tiny: |s|<~1 whp) and
fuses the denominator into the PV matmul via a ones-column on V.
"""
import os
import numpy as np
import ml_dtypes

E = 1024
H = 16
DH = 64
DFF = 4096
B, T = 2, 2048
NCORES = 8
TLOC = (B * T) // NCORES      # 512 tokens per core
NE = E // 128                 # 8 E-tiles
NQC = T // 512                # 4 q-chunks per batch
NKT = T // 128                # 16 key tiles per batch
EPS = 1e-5

_BUILT = None


def _build():
    import concourse.mybir as mybir
    import concourse.tile as tile
    from concourse import bacc

    f32 = mybir.dt.float32
    f32r = mybir.dt.float32r
    bf16 = mybir.dt.bfloat16
    AF = mybir.ActivationFunctionType

    nc = bacc.Bacc("TRN2", target_bir_lowering=False, debug=False,
                   num_devices=NCORES)

    # ---- external I/O ----
    xT_in = nc.dram_tensor("xT", [E, TLOC], f32, kind="ExternalInput")
    Wq_in = nc.dram_tensor("Wq", [E, E], f32, kind="ExternalInput")
    Wk_in = nc.dram_tensor("Wk", [E, E], f32, kind="ExternalInput")
    Wv_in = nc.dram_tensor("Wv", [E, E], f32, kind="ExternalInput")
    Wo_in = nc.dram_tensor("Wo", [E, E], bf16, kind="ExternalInput")
    W1_in = nc.dram_tensor("W1", [E, DFF], f32, kind="ExternalInput")
    W2_in = nc.dram_tensor("W2", [DFF, E], f32, kind="ExternalInput")
    vecs_in = nc.dram_tensor("vecs", [128, 80], f32, kind="ExternalInput")
    masks_in = nc.dram_tensor("masks", [4, 128, 512], bf16, kind="ExternalInput")
    out_ext = nc.dram_tensor("out", [E, TLOC], f32, kind="ExternalOutput")

    # ---- internal DRAM (A2A bounces) ----
    qa_i = nc.dram_tensor("qa_i", [NCORES, 128, TLOC], bf16)
    qa_o = nc.dram_tensor("qa_o", [NCORES, 128, TLOC], bf16)
    ka_i = nc.dram_tensor("ka_i", [NCORES, 128, TLOC], bf16)
    ka_o = nc.dram_tensor("ka_o", [NCORES, 128, TLOC], bf16)
    va_i = nc.dram_tensor("va_i", [NCORES, 4, 128, 130], bf16)
    va_o = nc.dram_tensor("va_o", [NCORES, 4, 128, 130], bf16)
    oa_ai = nc.dram_tensor("oa_ai", [NCORES, 128, TLOC], bf16)
    oa_ao = nc.dram_tensor("oa_ao", [NCORES, 128, TLOC], bf16)
    oa_bi = nc.dram_tensor("oa_bi", [NCORES, 128, TLOC], bf16)
    oa_bo = nc.dram_tensor("oa_bo", [NCORES, 128, TLOC], bf16)
    RG = [list(range(NCORES))]

    add = mybir.AluOpType.add
    mult = mybir.AluOpType.mult
    sub = mybir.AluOpType.subtract

    with tile.TileContext(nc) as tc:
        with (
            tc.tile_pool(name="const", bufs=1) as constp,
            tc.tile_pool(name="io", bufs=1) as io,
            tc.tile_pool(name="rows", bufs=1) as rows,
            tc.tile_pool(name="scr", bufs=1) as scr,
        ):
            # ---- constants ----
            vecs = constp.tile([128, 80], f32, tag="vecs")
            nc.sync.dma_start(out=vecs[:], in_=vecs_in[:])
            mask_sb = []
            for j in range(4):
                m = constp.tile([128, 512], bf16, tag=f"mask{j}")
                nc.sync.dma_start(out=m[:], in_=masks_in[j])
                mask_sb.append(m)
            ones_f32 = constp.tile([128, 16], f32, tag="ones_f32")
            nc.vector.memset(ones_f32[:], 1.0)
            ones_r = constp.tile([128, 1], f32r, tag="ones_r")
            nc.vector.tensor_copy(ones_r[:], ones_f32[:, 0:1])
            zero_bf = constp.tile([128, TLOC], bf16, tag="zero_bf")
            nc.vector.memset(zero_bf[:], 0.0)

            def vslice(col):          # [128,1] f32 slice of vecs
                return vecs[:, col:col + 1]

            # ---- load x^T ----
            xT = []
            for e in range(NE):
                t = io.tile([128, TLOC], f32r, tag=f"xT{e}", name=f"xT{e}")
                nc.sync.dma_start(out=t[:], in_=xT_in[128 * e:128 * e + 128, :].bitcast(f32r))
                xT.append(t)

            # ================= LayerNorm (T-layout) =================
            def layernorm(src, dstp, psp, dst_tag, gcol, becol):
                ps_sum = psp.tile([1, TLOC], f32, tag="ln_sum", bufs=1)
                ps_sq = psp.tile([1, TLOC], f32, tag="ln_sq", bufs=1)
                for e in range(NE):
                    sq = scr.tile([128, TLOC], f32r, tag="ln_sqt", bufs=2,
                                  name=f"sq_{dst_tag}{e}")
                    nc.vector.tensor_mul(sq[:], src[e][:].bitcast(f32),
                                         src[e][:].bitcast(f32))
                    nc.tensor.matmul(ps_sum[:], ones_r[:], src[e][:],
                                     start=(e == 0), stop=(e == NE - 1))
                    nc.tensor.matmul(ps_sq[:], ones_r[:], sq[:],
                                     start=(e == 0), stop=(e == NE - 1))
                mean_r = rows.tile([1, TLOC], f32, tag="row", bufs=6,
                                   name=f"mean_{dst_tag}")
                nc.vector.tensor_scalar_mul(mean_r[:], ps_sum[:], 1.0 / E)
                m2 = rows.tile([1, TLOC], f32, tag="row", bufs=6, name=f"m2_{dst_tag}")
                nc.vector.tensor_mul(m2[:], mean_r[:], mean_r[:])
                vpe = rows.tile([1, TLOC], f32, tag="row", bufs=6, name=f"vpe_{dst_tag}")
                # vpe = (ps_sq/E - m2) + EPS  in two steps
                nc.vector.scalar_tensor_tensor(vpe[:], ps_sq[:], 1.0 / E, m2[:],
                                               mult, sub)
                nc.vector.tensor_scalar_add(vpe[:], vpe[:], EPS)
                rec = rows.tile([1, TLOC], f32, tag="row", bufs=6, name=f"rec_{dst_tag}")
                nc.vector.reciprocal(rec[:], vpe[:])
                a_r = rows.tile([1, TLOC], f32, tag="row", bufs=6, name=f"a_{dst_tag}")
                nc.scalar.activation(a_r[:], rec[:], AF.Sqrt)
                b_r = rows.tile([1, TLOC], f32, tag="row", bufs=6, name=f"b_{dst_tag}")
                nc.vector.tensor_mul(b_r[:], mean_r[:], a_r[:])
                nc.vector.tensor_scalar_mul(b_r[:], b_r[:], -1.0)
                a_bc = rows.tile([128, TLOC], f32, tag="bc", bufs=2,
                                 name=f"abc_{dst_tag}")
                nc.gpsimd.partition_broadcast(a_bc[:], a_r[:])
                b_bc = rows.tile([128, TLOC], f32, tag="bc", bufs=2,
                                 name=f"bbc_{dst_tag}")
                nc.gpsimd.partition_broadcast(b_bc[:], b_r[:])
                dst = []
                for e in range(NE):
                    t1 = scr.tile([128, TLOC], f32, tag="ln_t1", bufs=2,
                                  name=f"t1_{dst_tag}{e}")
                    nc.vector.tensor_mul(t1[:], src[e][:].bitcast(f32), a_bc[:])
                    nc.vector.tensor_add(t1[:], t1[:], b_bc[:])
                    h = dstp.tile([128, TLOC], f32r, tag=f"{dst_tag}{e}",
                                  name=f"{dst_tag}{e}")
                    nc.vector.tensor_scalar(h[:], t1[:], vslice(gcol + e),
                                            vslice(becol + e), mult, add)
                    dst.append(h)
                return dst

            # ================= LN1 + QKV =================
            with (
                tc.tile_pool(name="pA", bufs=1) as pA,
                tc.tile_pool(name="pW", bufs=1) as pW,
            ):
                with tc.tile_pool(name="psLN1", bufs=1, space="PSUM") as psLN1:
                    hT = layernorm(xT, pA, psLN1, "hT", 0, 8)

                def qk_weights(W_dram, dest, psQKV):
                    w_all = pW.tile([128, NE * E], f32r, tag="wbig", bufs=2,
                                    name=f"w_{dest.name}")
                    for e in range(NE):
                        nc.sync.dma_start(
                            out=w_all[:, E * e:E * e + E],
                            in_=W_dram[128 * e:128 * e + 128, :].bitcast(f32r))
                    for gp in range(4):          # pairs of head-groups
                        ps = psQKV.tile([128, 1024], f32, tag="qk_ps", bufs=2,
                                        name=f"qkps_{dest.name}{gp}")
                        for e in range(NE):
                            for i in range(2):
                                g = 2 * gp + i
                                nc.tensor.matmul(
                                    ps[:, 512 * i:512 * i + 512],
                                    w_all[:, E * e + 128 * g:E * e + 128 * g + 128],
                                    hT[e][:], start=(e == 0), stop=(e == NE - 1))
                        qt = scr.tile([128, 1024], bf16, tag="qkt", bufs=3,
                                      name=f"qt_{dest.name}{gp}")
                        nc.vector.tensor_copy(qt[:], ps[:])
                        for i in range(2):
                            nc.sync.dma_start(out=dest[2 * gp + i],
                                              in_=qt[:, 512 * i:512 * i + 512])

                with tc.tile_pool(name="psQKV", bufs=1, space="PSUM") as psQKV:
                    qk_weights(Wq_in, qa_i, psQKV)
                    nc.gpsimd.collective_compute(
                        "AllToAll", mybir.AluOpType.bypass,
                        ins=[qa_i[:]], outs=[qa_o[:]], replica_groups=RG)
                    qk_weights(Wk_in, ka_i, psQKV)
                    nc.gpsimd.collective_compute(
                        "AllToAll", mybir.AluOpType.bypass,
                        ins=[ka_i[:]], outs=[ka_o[:]], replica_groups=RG)

                    # V natural [tok, 16*65 aug]; lhsT = h^T token-slices
                    wv_all = pW.tile([128, NE * E], f32r, tag="wbig", bufs=2,
                                     name="w_v")
                    for e in range(NE):
                        nc.sync.dma_start(
                            out=wv_all[:, E * e:E * e + E],
                            in_=Wv_in[128 * e:128 * e + 128, :].bitcast(f32r))
                    for t in range(4):
                        vaug = scr.tile([128, 16 * 65], bf16, tag="vaug", bufs=2,
                                        name=f"vaug{t}")
                        ps = psQKV.tile([128, 1024], f32, tag="v_ps", bufs=2,
                                        name=f"vps{t}")
                        for e in range(NE):
                            for half in range(2):
                                nc.tensor.matmul(
                                    ps[:, 512 * half:512 * half + 512],
                                    hT[e][:, 128 * t:128 * t + 128],
                                    wv_all[:, E * e + 512 * half:E * e + 512 * half + 512],
                                    start=(e == 0), stop=(e == NE - 1))
                        dst = vaug[:].rearrange("p (h c) -> p h c", c=65)[:, :, 0:64]
                        nc.vector.tensor_copy(
                            dst, ps[:].rearrange("p (h c) -> p h c", c=64))
                        ov = vaug[:].rearrange("p (h c) -> p h c", c=65)[:, :, 64].squeeze()
                        nc.vector.tensor_copy(ov, ones_f32[:])
                        for d in range(NCORES):
                            nc.sync.dma_start(out=va_i[d, t],
                                              in_=vaug[:, 130 * d:130 * d + 130])
                    nc.gpsimd.collective_compute(
                        "AllToAll", mybir.AluOpType.bypass,
                        ins=[va_i[:]], outs=[va_o[:]], replica_groups=RG)

            # Wo prefetch pool spans attention + proj
            with (
                tc.tile_pool(name="pWo", bufs=1) as pWo,
                tc.tile_pool(name="pC", bufs=1) as pC,
            ):
                wo_all = pWo.tile([128, NE * E], bf16, tag="wo", bufs=1, name="w_o")
                for e in range(NE):
                    nc.sync.dma_start(out=wo_all[:, E * e:E * e + E],
                                      in_=Wo_in[128 * e:128 * e + 128, :])

                # ============= attention (2 local heads, all tokens) ====
                with (
                    tc.tile_pool(name="pB", bufs=1) as pB,
                    tc.tile_pool(name="psATT", bufs=1, space="PSUM") as psATT,
                ):
                    att_q = pB.tile([128, B * T], bf16, tag="att_q")
                    att_k = pB.tile([128, B * T], bf16, tag="att_k")
                    for s in range(NCORES):
                        nc.sync.dma_start(out=att_q[:, TLOC * s:TLOC * s + TLOC],
                                          in_=qa_o[s])
                        nc.sync.dma_start(out=att_k[:, TLOC * s:TLOC * s + TLOC],
                                          in_=ka_o[s])
                    v_att = []
                    for gkt in range(2 * NKT):
                        vt = pB.tile([128, 130], bf16, tag=f"v_att{gkt}",
                                     name=f"v_att{gkt}")
                        nc.sync.dma_start(out=vt[:],
                                          in_=va_o[gkt // 4, gkt % 4])
                        v_att.append(vt)

                    O_pack = [pB.tile([128, T], bf16, tag=f"O_pack{bb_}",
                                      name=f"O_pack{bb_}")
                              for bb_ in range(B)]
                    for b in range(B):
                        for qc in range(NQC):
                            gc = NQC * b + qc
                            nkt = 4 * qc + 4
                            ps_os = [psATT.tile([65, 512], f32, tag="o_ps",
                                                 bufs=4, name=f"pso_{gc}_{h}")
                                     for h in range(2)]
                            for kt in range(nkt):
                                gkt = NKT * b + kt
                                ps_s = psATT.tile([128, 1024], f32, tag="s_ps",
                                                  bufs=2, name=f"pss_{gc}_{kt}")
                                for h in range(2):
                                    nc.tensor.matmul(
                                        ps_s[:, 512 * h:512 * h + 512],
                                        att_k[64 * h:64 * h + 64,
                                              128 * gkt:128 * gkt + 128],
                                        att_q[64 * h:64 * h + 64,
                                              512 * gc:512 * gc + 512],
                                        start=True, stop=True)
                                p = scr.tile([128, 1024], bf16, tag="p_t", bufs=3,
                                             name=f"p_{gc}_{kt}")
                                nc.scalar.activation(p[:], ps_s[:], AF.Exp)
                                if kt >= 4 * qc:
                                    for h in range(2):
                                        nc.vector.tensor_mul(
                                            p[:, 512 * h:512 * h + 512],
                                            p[:, 512 * h:512 * h + 512],
                                            mask_sb[kt - 4 * qc][:])
                                for h in range(2):
                                    nc.tensor.matmul(
                                        ps_os[h][:],
                                        v_att[gkt][:, 65 * h:65 * h + 65],
                                        p[:, 512 * h:512 * h + 512],
                                        start=(kt == 0), stop=(kt == nkt - 1))
                            for h in range(2):
                                recip = rows.tile([1, 512], f32, tag="row", bufs=6,
                                                  name=f"recip_{gc}_{h}")
                                nc.vector.reciprocal(recip[:], ps_os[h][64:65, :])
                                rbc = rows.tile([64, 512], f32, tag="rbc", bufs=2,
                                                name=f"rbc_{gc}_{h}")
                                nc.gpsimd.partition_broadcast(rbc[:], recip[:])
                                nc.vector.tensor_mul(
                                    O_pack[b][64 * h:64 * h + 64,
                                              512 * qc:512 * qc + 512],
                                    ps_os[h][0:64, :], rbc[:])

                        # ===== batch-split AllToAll #2 =====
                        if qc == NQC - 1:
                            src_buf = oa_ai if b == 0 else oa_bi
                            dst_buf = oa_ao if b == 0 else oa_bo
                            for d in range(NCORES):
                                if d // 4 == b:
                                    nc.sync.dma_start(
                                        out=src_buf[d],
                                        in_=O_pack[b][:, TLOC * (d % 4):
                                                      TLOC * (d % 4) + TLOC])
                                else:
                                    nc.sync.dma_start(out=src_buf[d], in_=zero_bf[:])
                            nc.gpsimd.collective_compute(
                                "AllToAll", mybir.AluOpType.bypass,
                                ins=[src_buf[:]], outs=[dst_buf[:]],
                                replica_groups=RG)
                O_la, O_lb = [], []
                for s in range(NCORES):
                    ta = pC.tile([128, TLOC], bf16, tag=f"O_la{s}", name=f"O_la{s}")
                    nc.sync.dma_start(out=ta[:], in_=oa_ao[s])
                    O_la.append(ta)
                    tb = pC.tile([128, TLOC], bf16, tag=f"O_lb{s}", name=f"O_lb{s}")
                    nc.sync.dma_start(out=tb[:], in_=oa_bo[s])
                    O_lb.append(tb)

                # ===== proj + residual (a-half then b-half, by linearity) =====
                x2T = []
                with tc.tile_pool(name="psPROJ", bufs=1, space="PSUM") as psPROJ:
                    for et in range(NE):
                        ps = psPROJ.tile([128, TLOC], f32, tag="proj_ps", bufs=3,
                                         name=f"projps{et}")
                        for s in range(NCORES):
                            nc.tensor.matmul(
                                ps[:],
                                wo_all[:, E * s + 128 * et:E * s + 128 * et + 128],
                                O_la[s][:], start=(s == 0), stop=False)
                        for s in range(NCORES):
                            nc.tensor.matmul(
                                ps[:],
                                wo_all[:, E * s + 128 * et:E * s + 128 * et + 128],
                                O_lb[s][:], start=False, stop=(s == NCORES - 1))
                        t = io.tile([128, TLOC], f32r, tag=f"x2T{et}", name=f"x2T{et}")
                        nc.vector.scalar_tensor_tensor(t[:], ps[:], vslice(32 + et),
                                                       xT[et][:].bitcast(f32), add, add)
                        x2T.append(t)

            # ================= LN2 + FFN =================
            with (
                tc.tile_pool(name="pD", bufs=1) as pD,
                tc.tile_pool(name="pWf", bufs=1) as pWf,
            ):
                with tc.tile_pool(name="psLN2", bufs=1, space="PSUM") as psLN2:
                    h2T = layernorm(x2T, pD, psLN2, "h2T", 16, 24)

                with tc.tile_pool(name="psFFN", bufs=1, space="PSUM") as psFFN:
                    ff = []
                    for dblk in range(8):
                        ps_l = [psFFN.tile([128, TLOC], f32, tag=f"ff_ps{i}", bufs=1,
                                           name=f"ff_ps_{dblk}_{i}")
                                for i in range(4)]
                        for e in range(NE):
                            w1t = pWf.tile([128, 512], f32r, tag="w1", bufs=3,
                                           name=f"w1_{dblk}_{e}")
                            nc.sync.dma_start(
                                out=w1t[:],
                                in_=W1_in[128 * e:128 * e + 128,
                                          512 * dblk:512 * dblk + 512].bitcast(f32r))
                            for i in range(4):
                                nc.tensor.matmul(ps_l[i][:], w1t[:, 128 * i:128 * i + 128],
                                                 h2T[e][:], start=(e == 0),
                                                 stop=(e == NE - 1))
                        for i in range(4):
                            d = 4 * dblk + i
                            f = pD.tile([128, TLOC], f32r, tag=f"ff{d}", name=f"ff{d}")
                            nc.scalar.activation(f[:], ps_l[i][:], AF.Relu,
                                                 bias=vslice(48 + d))
                            ff.append(f)

                    for group in ([0, 1, 2, 3], [4, 5, 6, 7]):
                        ps2 = {et: psFFN.tile([128, TLOC], f32, tag=f"ps2_{et % 4}",
                                              bufs=1, name=f"ps2t_{et}")
                               for et in group}
                        c0, c1 = 128 * group[0], 128 * (group[-1] + 1)
                        for d in range(32):
                            w2t = pWf.tile([128, c1 - c0], f32r, tag="w2", bufs=3,
                                           name=f"w2_{group[0]}_{d}")
                            nc.sync.dma_start(
                                out=w2t[:],
                                in_=W2_in[128 * d:128 * d + 128, c0:c1].bitcast(f32r))
                            for i, et in enumerate(group):
                                nc.tensor.matmul(ps2[et][:], w2t[:, 128 * i:128 * i + 128],
                                                 ff[d][:], start=(d == 0), stop=(d == 31))
                        for et in group:
                            o_sb = scr.tile([128, TLOC], f32, tag="osb", bufs=3,
                                            name=f"osb{et}")
                            nc.vector.scalar_tensor_tensor(o_sb[:], ps2[et][:],
                                                           vslice(40 + et),
                                                           x2T[et][:].bitcast(f32),
                                                           add, add)
                            nc.sync.dma_start(out=out_ext[128 * et:128 * et + 128, :],
                                              in_=o_sb[:])

    nc.finalize()
    return nc


def _get_built():
    global _BUILT
    if _BUILT is None:
        _BUILT = _build()
    return _BUILT


def _pack_vec(v):
    return np.ascontiguousarray(v.reshape(-1, 128).T.astype(np.float32))


def _host_inputs(x, Wq, Wk, Wv, Wo, bo, W1, b1, W2, b2, g1, be1, g2, be2):
    xf = np.asarray(x, dtype=np.float32).reshape(B * T, E)
    scale = np.float32(E) ** np.float32(-0.5)
    Wq_s = (np.asarray(Wq, np.float32) * scale).astype(np.float32)
    vecs = np.concatenate([
        _pack_vec(np.asarray(g1, np.float32)),
        _pack_vec(np.asarray(be1, np.float32)),
        _pack_vec(np.asarray(g2, np.float32)),
        _pack_vec(np.asarray(be2, np.float32)),
        _pack_vec(np.asarray(bo, np.float32)),
        _pack_vec(np.asarray(b2, np.float32)),
        _pack_vec(np.asarray(b1, np.float32)),
    ], axis=1)  # [128, 80]
    k = np.arange(128)[:, None]
    q = np.arange(512)[None, :]
    masks = np.stack([(q >= 128 * j + k).astype(ml_dtypes.bfloat16)
                      for j in range(4)])
    common = {
        "Wq": Wq_s, "Wk": np.asarray(Wk, np.float32), "Wv": np.asarray(Wv, np.float32),
        "Wo": np.asarray(Wo, np.float32).astype(ml_dtypes.bfloat16),
        "W1": np.asarray(W1, np.float32),
        "W2": np.asarray(W2, np.float32), "vecs": vecs, "masks": masks,
    }
    in_maps = []
    for c in range(NCORES):
        m = dict(common)
        m["xT"] = np.ascontiguousarray(xf[TLOC * c:TLOC * (c + 1)].T)
        in_maps.append(m)
    return in_maps


def run_spmd(inputs, trace=False):
    from concourse.bass_utils import run_bass_kernel_spmd
    nc = _get_built()
    in_maps = _host_inputs(**inputs)
    res = run_bass_kernel_spmd(nc, in_maps, list(range(NCORES)), trace=trace)
    outs = [res.results[c]["out"] for c in range(NCORES)]
    y = np.concatenate([o.T for o in outs], axis=0).reshape(B, T, E)
    return np.ascontiguousarray(y.astype(np.float32)), res


def kernel(**inputs):
    y, _ = run_spmd(inputs, trace=False)
    return y

